# revision 1
# baseline (speedup 1.0000x reference)
"""Trainium2 Bass kernel for nn_Convnet_81862076661945 (topk_masking).

Pipeline (per the reference nn.Module):
  - X [3231, 256] f32 is sliced into 8 overlapping time sections [431, 256]
    (stride 400).
  - Section s is convolved (VALID) with W[s] [128, 1, 32, 16] -> potentials
    [128, 400, 241].
  - spikes = potentials >= 15.0; max-pool over (400, 16) windows -> [128, 1, 15]
  - A stacked k-winner reduction over the 8 sections produces a single int32
    channel index (or -1).

Sharding: section-parallel — core s owns section s (tensor core does the
conv at M=128 channels = full PE width). The tiny pooled maps [128, 15] are
all-gathered across the 8 cores and every core redundantly computes the
final winner on-device.

Conv-as-matmul mapping (per core):
  Contraction K = 128 = (4 freq-shift group dfc) x (32 time taps dt), with 4
  PSUM-accumulated matmuls g covering freq taps df = 4g + dfc.  The rhs
  im2col tile for a batch of output times is materialized by a single
  strided DMA from a host-prepared tensor xsh[dfc, r, k] = X_sec[r, dfc+k]
  (4 freq-shifted copies of the section), so each partition row is a fully
  contiguous read.  Weights are host-packed to lhsT[g][dfc*32+dt, c].
"""

import sys

if "/opt/trn_rl_repo" not in sys.path:
    sys.path.insert(0, "/opt/trn_rl_repo")

import numpy as np
import ml_dtypes

import concourse.bass as bass
import concourse.bacc as bacc
import concourse.mybir as mybir
import concourse.tile as tile
from concourse.bass_utils import run_bass_kernel_spmd
import bass_rust

# problem constants (hardcoded per harness contract)
N_SECTIONS, N_CHANNELS = 8, 128
KT, KF = 32, 16
LPOST = 400                       # output times per section
LPRE = KT + LPOST - 1             # 431 input rows per section
SECTION_DISTANCE = 400
N_TIMESTEPS, FREQ = 3231, 256
THRESHOLD = 15.0
FOUT = FREQ - KF + 1              # 241 output freqs
FP = FOUT // KF                   # 15 pooled freqs
NDFC = 4                          # freq shifts baked into partitions
NG = KF // NDFC                   # 4 PSUM-accumulated matmuls
T_BATCH = 8                       # output times per im2col DMA
N_BATCH = LPOST // T_BATCH        # 50
T_PAIR = 2                        # output times per PSUM bank (2*241 <= 512)

BF16 = mybir.dt.bfloat16
F32 = mybir.dt.float32
I32 = mybir.dt.int32
OP = mybir.AluOpType


def _ap(handle, offset, dims):
    """Arbitrary strided access pattern on a tensor handle."""
    return bass_rust.AP(handle, offset, [list(d) for d in dims])


def build_nc():
    nc = bacc.Bacc(num_devices=N_SECTIONS)

    xsh = nc.dram_tensor("xsh", [NDFC, LPRE, FREQ], BF16, kind="ExternalInput")
    wt = nc.dram_tensor("wt", [NG, 128, 128], BF16, kind="ExternalInput")
    out = nc.dram_tensor("out", [1, 1], I32, kind="ExternalOutput")
    pool_dbg = nc.dram_tensor("pool_dbg", [N_CHANNELS, FP], F32, kind="ExternalOutput")
    cc_in = nc.dram_tensor("cc_in", [N_CHANNELS, FP], F32)
    cc_out = nc.dram_tensor(
        "cc_out", [N_SECTIONS, N_CHANNELS, FP], F32, addr_space="Shared"
    )

    with tile.TileContext(nc) as tc:
        with (
            tc.tile_pool(name="wp", bufs=1) as wp,
            tc.tile_pool(name="xp", bufs=8) as xp,
            tc.tile_pool(name="pp", bufs=6, space="PSUM") as pp,
            tc.tile_pool(name="pf", bufs=1, space="PSUM") as pf,
            tc.tile_pool(name="mp", bufs=1) as mpool,
        ):
            # ---- weights: SBUF [p=(dfc,dt)=128, (g, c)] ----
            wtile = wp.tile([128, NG * 128], BF16)
            nc.sync.dma_start(
                out=wtile[:].rearrange("p (g c) -> p g c", g=NG),
                in_=wt[:].rearrange("g p c -> p g c"),
            )

            # ---- per-pair windowed maxes land in independent slots ----
            n_pairs = LPOST // T_PAIR
            slot = T_PAIR * FP
            macc = mpool.tile([128, n_pairs * slot], F32)

            xsh_h = xsh[:].tensor

            for b in range(N_BATCH):
                t0 = b * T_BATCH
                xr = xp.tile([128, T_BATCH * FREQ], BF16)
                # partition (dfc, dt) row tt holds xsh[dfc, t0+tt+dt, 0:256]
                src = _ap(
                    xsh_h,
                    t0 * FREQ,
                    [
                        (LPRE * FREQ, NDFC),   # dfc  (partition, outer)
                        (FREQ, KT),            # dt   (partition, inner)
                        (FREQ, T_BATCH),       # tt   (free)
                        (1, FREQ),             # k    (free)
                    ],
                )
                # dst iterates p=(dfc,dt) then 2048 contiguous elements — the
                # src dims (dfc, dt, tt, k) match that order exactly.
                nc.sync.dma_start(out=xr[:], in_=src)

                xr3 = xr[:].rearrange("p (tt k) -> p tt k", tt=T_BATCH)
                for pr in range(T_BATCH // T_PAIR):
                    ps = pp.tile([128, T_PAIR * FOUT], F32)
                    for g in range(NG):
                        rhs = xr3[:, T_PAIR * pr : T_PAIR * (pr + 1),
                                  4 * g : 4 * g + FOUT]
                        lhsT = wtile[:, g * 128 : (g + 1) * 128]
                        nc.tensor.matmul(
                            ps[:], lhsT, rhs, start=(g == 0), stop=(g == NG - 1)
                        )
                    # windowed max [128, 2, 15, 16] -> this pair's slot
                    pair = b * (T_BATCH // T_PAIR) + pr
                    ps4 = ps[:].rearrange("p (a k) -> p a k", a=T_PAIR)[
                        :, :, : FP * KF
                    ].rearrange("p a (q w) -> p a q w", w=KF)
                    dst = macc[:, pair * slot : (pair + 1) * slot].rearrange(
                        "p (a q) -> p a q", a=T_PAIR
                    )
                    nc.vector.tensor_reduce(
                        dst, ps4, axis=mybir.AxisListType.X, op=OP.max
                    )

            # ---- final max over all 400 output times ----
            mpt = mpool.tile([128, FP], F32)
            mview = macc[:].rearrange(
                "p (pair a q) -> p q pair a", pair=n_pairs, a=T_PAIR
            )
            nc.vector.tensor_reduce(
                mpt[:], mview, axis=mybir.AxisListType.XY, op=OP.max
            )
            nc.sync.dma_start(out=pool_dbg[:], in_=mpt[:])
            nc.sync.dma_start(out=cc_in[:], in_=mpt[:])

            # ---- all-gather pooled max-pot maps across the 8 cores ----
            nc.gpsimd.collective_compute(
                "AllGather",
                OP.bypass,
                replica_groups=[list(range(N_SECTIONS))],
                ins=[cc_in[:]],
                outs=[cc_out[:]],
            )

            # ---- final k-winner logic (identical on every core) ----
            # G[c, fp, s] = maxpot of section s
            gt = mpool.tile([128, FP * N_SECTIONS], F32)
            gsrc = _ap(
                cc_out[:].tensor,
                0,
                [
                    (FP, N_CHANNELS),            # c (partition)
                    (1, FP),                     # fp (free)
                    (N_CHANNELS * FP, N_SECTIONS),  # s (free, innermost)
                ],
            )
            gt3 = gt[:].rearrange("p (q s) -> p q s", s=N_SECTIONS)
            nc.sync.dma_start(out=gt3, in_=gsrc)

            spk = mpool.tile([128, FP * N_SECTIONS], F32)
            spk3 = spk[:].rearrange("p (q s) -> p q s", s=N_SECTIONS)
            nc.vector.tensor_single_scalar(spk3, gt3, THRESHOLD, OP.is_ge)

            n_t = mpool.tile([128, FP], F32)
            nc.vector.tensor_reduce(
                n_t[:], spk3, axis=mybir.AxisListType.X, op=OP.add
            )
            # earliest = clip(8 - n, 0, 7) = min(8 - n, 7)  (n in [0, 8])
            e_t = mpool.tile([128, FP], F32)
            nc.vector.tensor_scalar(
                e_t[:], n_t[:], float(N_SECTIONS), -1.0, OP.subtract, OP.mult
            )
            nc.vector.tensor_scalar_min(e_t[:], e_t[:], float(N_SECTIONS - 1))

            # values[c,fp] = spk[e[c,fp]][c,fp]  via sum_s spk_s * (e == s)
            val = mpool.tile([128, FP], F32)
            nc.vector.memset(val[:], 0.0)
            tmp = mpool.tile([128, FP], F32)
            for s in range(N_SECTIONS):
                nc.vector.scalar_tensor_tensor(
                    tmp[:], e_t[:], float(s), spk3[:, :, s], OP.is_equal, OP.mult
                )
                nc.vector.tensor_tensor(val[:], val[:], tmp[:], OP.add)

            # ---- helpers for cross-partition reduce via PE ----
            # iomat[p, j] = p - j  (f32 exact for |v| <= 127); identity = (iomat == 0)
            iomat = mpool.tile([128, 128], F32)
            nc.gpsimd.iota(
                iomat[:], [[-1, 128]], base=0, channel_multiplier=1,
                allow_small_or_imprecise_dtypes=True,
            )
            idn = mpool.tile([128, 128], F32)
            nc.vector.tensor_single_scalar(idn[:], iomat[:], 0.0, OP.is_equal)
            ones1 = mpool.tile([1, 128], F32)
            nc.vector.memset(ones1[:], 1.0)

            def col_to_row(col_ap, tag):
                """[128,1] SBUF -> [1,128] SBUF via matmul with identity."""
                pst = pf.tile([1, 128], F32, tag="pcc")
                nc.tensor.matmul(pst[:], col_ap, idn[:], start=True, stop=True)
                row = mpool.tile([1, 128], F32, tag=f"row_{tag}")
                nc.vector.tensor_copy(row[:], pst[:])
                return row

            def bcast_scalar(s11, tag):
                """[1,1] SBUF (partition 0) -> [128,1] SBUF."""
                psb = pf.tile([128, 1], F32, tag="pcc")
                nc.tensor.matmul(psb[:], ones1[:], s11, start=True, stop=True)
                full = mpool.tile([128, 1], F32, tag=f"bc_{tag}")
                nc.vector.tensor_copy(full[:], psb[:])
                return full

            # v = 8 * max(spk * values_broadcast) = 8 * max(values * min(n,1))
            nmin = mpool.tile([128, FP], F32)
            nc.vector.tensor_scalar_min(nmin[:], n_t[:], 1.0)
            q_t = mpool.tile([128, FP], F32)
            nc.vector.tensor_tensor(q_t[:], val[:], nmin[:], OP.mult)
            rq = mpool.tile([128, 1], F32)
            nc.vector.tensor_reduce(rq[:], q_t[:], axis=mybir.AxisListType.X, op=OP.max)
            rq_row = col_to_row(rq[:], "rq")
            q1 = mpool.tile([1, 1], F32)
            nc.vector.tensor_reduce(q1[:], rq_row[:], axis=mybir.AxisListType.X, op=OP.max)
            v8_all = bcast_scalar(q1[:], "v8")
            nc.vector.tensor_scalar_mul(v8_all[:], v8_all[:], float(N_SECTIONS))

            # total = (values + v8) * n
            tot = mpool.tile([128, FP], F32)
            nc.vector.scalar_tensor_tensor(
                tot[:], val[:], v8_all[:], n_t[:], OP.add, OP.mult
            )

            # global max M and first row achieving it
            rmax = mpool.tile([128, 1], F32)
            nc.vector.tensor_reduce(
                rmax[:], tot[:], axis=mybir.AxisListType.X, op=OP.max
            )
            rm_row = col_to_row(rmax[:], "rm")
            m1 = mpool.tile([1, 1], F32)
            nc.vector.tensor_reduce(m1[:], rm_row[:], axis=mybir.AxisListType.X, op=OP.max)
            gmax_all = bcast_scalar(m1[:], "gm")

            elig = mpool.tile([128, 1], F32)
            nc.vector.tensor_tensor(elig[:], rmax[:], gmax_all[:], OP.is_equal)
            # idx = elig ? c : 1e9 ; feat = min over partitions = -max(-idx)
            iof = iomat[:, 0:1]  # iomat[p, 0] = p
            a_t = mpool.tile([128, 1], F32)
            nc.vector.tensor_tensor(a_t[:], elig[:], iof, OP.mult)
            b_t = mpool.tile([128, 1], F32)
            nc.vector.tensor_scalar(b_t[:], elig[:], 1e9, -1e9, OP.mult, OP.add)
            nidx = mpool.tile([128, 1], F32)
            nc.vector.tensor_tensor(nidx[:], b_t[:], a_t[:], OP.subtract)
            # nidx = (elig*1e9 - 1e9) - elig*c = -(idx); max(nidx) = -min(idx)
            ni_row = col_to_row(nidx[:], "ni")
            nf1 = mpool.tile([1, 1], F32)
            nc.vector.tensor_reduce(
                nf1[:], ni_row[:], axis=mybir.AxisListType.X, op=OP.max
            )
            feat1 = mpool.tile([1, 1], F32)
            nc.vector.tensor_scalar_mul(feat1[:], nf1[:], -1.0)

            # ans = (M > 0) ? feat : -1  == feat*gtz + (gtz - 1)
            gtz = mpool.tile([1, 1], F32)
            nc.vector.tensor_single_scalar(gtz[:], m1[:], 0.0, OP.is_gt)
            c1 = mpool.tile([1, 1], F32)
            nc.vector.tensor_tensor(c1[:], feat1[:], gtz[:], OP.mult)
            c2 = mpool.tile([1, 1], F32)
            nc.vector.tensor_scalar_sub(c2[:], gtz[:], 1.0)
            ansf = mpool.tile([1, 1], F32)
            nc.vector.tensor_tensor(ansf[:], c1[:], c2[:], OP.add)
            ansi = mpool.tile([1, 1], I32)
            nc.vector.tensor_copy(ansi[:], ansf[:])
            nc.sync.dma_start(out=out[:], in_=ansi[:])

    nc.compile()
    return nc


def prep_inputs(X, W):
    """Host-side sharding + layout packing. Returns in_maps for 8 cores."""
    X = np.asarray(X, dtype=np.float32)
    W = np.asarray(W, dtype=np.float32)
    in_maps = []
    for s in range(N_SECTIONS):
        xs = X[s * SECTION_DISTANCE : s * SECTION_DISTANCE + LPRE]  # [431, 256]
        xsh = np.zeros((NDFC, LPRE, FREQ), dtype=np.float32)
        for dfc in range(NDFC):
            xsh[dfc, :, : FREQ - dfc] = xs[:, dfc:]
        # wt[g, dfc*32+dt, c] = W[s, c, 0, dt, 4g+dfc]
        wts = np.ascontiguousarray(
            W[s, :, 0].transpose(2, 1, 0)  # [df, dt, c]
        ).reshape(NG, NDFC, KT, N_CHANNELS).reshape(NG, 128, N_CHANNELS)
        in_maps.append(
            {
                "xsh": xsh.astype(ml_dtypes.bfloat16),
                "wt": wts.astype(ml_dtypes.bfloat16),
            }
        )
    return in_maps


_NC_CACHE = {}


def run(X, W, trace=False, **kwargs):
    if "nc" not in _NC_CACHE:
        _NC_CACHE["nc"] = build_nc()
    nc = _NC_CACHE["nc"]
    in_maps = prep_inputs(X, W)
    res = run_bass_kernel_spmd(
        nc, in_maps, core_ids=list(range(N_SECTIONS)), trace=trace, **kwargs
    )
    return np.int32(res.results[0]["out"][0, 0]), res


def kernel(X, W):
    ans, _ = run(X, W)
    return ans


if __name__ == "__main__":
    X = np.random.rand(N_TIMESTEPS, FREQ).astype(np.float32) * 0.073
    W = (0.8 + 0.05 * np.random.randn(N_SECTIONS, N_CHANNELS, 1, KT, KF)).astype(
        np.float32
    )
    print(kernel(X, W))



# revision 4
# speedup vs baseline: 1.8874x; 1.8874x over previous
"""Trainium2 Bass kernel for nn_Convnet_81862076661945 (topk_masking).

Pipeline (per the reference nn.Module):
  - X [3231, 256] f32 is sliced into 8 overlapping time sections [431, 256]
    (stride 400).
  - Section s is convolved (VALID) with W[s] [128, 1, 32, 16] -> potentials
    [128, 400, 241].
  - spikes = potentials >= 15.0; max-pool over (400, 16) windows -> [128, 1, 15]
  - A stacked k-winner reduction over the 8 sections produces a single int32
    channel index (or -1).

Sharding: section-parallel - core s owns section s. The tiny pooled binary
maps [128, 15] are all-gathered and every core redundantly computes the final
winner on-device.

Conv mapping (per core): fp8(e4m3) DoubleRow matmuls. Contraction 512 taps =
2 h-groups x (2 i-groups x 128 rows (dfc x dt)) where freq tap
df = 4*(2h+i) + dfc. Per pair of output times: 2 PSUM-accumulated DoubleRow
matmuls (each contracting 256) with 512 output columns (2 x 256, cols f>=241
are garbage and never read). The im2col rhs tile is a single strided DMA from
one fp8 copy of the section: partition (dfc, dt) holds a contiguous 2080-byte
run starting at x[t0 + dt, dfc].

Pooling: window max over (t, 16-freq) windows. Per pair, either the vector
engine reduces straight from PSUM, or the scalar engine copies PSUM -> SBUF
bf16 and the vector engine reduces at 4x. Partial maxes accumulate in a
q-major [128, 15*400] bf16 buffer; one 4x reduce collapses it, then one
threshold yields the binary spike map.

Final winner: spike maps all-gathered via collective; per-channel stats
computed with ~30 small vector ops, cross-partition maxima via 3 PE
transposes. total' = n*(val+8); feat from a packed (256*max - channel) trick.
"""

import sys

if "/opt/trn_rl_repo" not in sys.path:
    sys.path.insert(0, "/opt/trn_rl_repo")

import numpy as np
import ml_dtypes

import concourse.bass as bass
import concourse.bacc as bacc
import concourse.mybir as mybir
import concourse.tile as tile
from concourse.bass_utils import run_bass_kernel_spmd
import bass_rust

# problem constants (hardcoded per harness contract)
N_SECTIONS, N_CHANNELS = 8, 128
KT, KF = 32, 16
LPOST = 400                       # output times per section
LPRE = KT + LPOST - 1             # 431 input rows per section
SECTION_DISTANCE = 400
N_TIMESTEPS, FREQ = 3231, 256
THRESHOLD = 15.0
FOUT = FREQ - KF + 1              # 241 output freqs
FP = FOUT // KF                   # 15 pooled freqs
NDFC = 4                          # freq shifts baked into partitions
T_BATCH = 8                       # output times per im2col DMA
N_BATCH = LPOST // T_BATCH        # 50
T_PAIR = 2                        # output times per PSUM bank
PAIRS_PER_BATCH = T_BATCH // T_PAIR
N_PAIRS = LPOST // T_PAIR         # 200
XCOLS = T_BATCH * FREQ + 32       # im2col tile cols (pad for group shifts)

FP8 = mybir.dt.float8e4
BF16 = mybir.dt.bfloat16
F32 = mybir.dt.float32
I32 = mybir.dt.int32
OP = mybir.AluOpType
DR = mybir.MatmulPerfMode.DoubleRow
AF = mybir.ActivationFunctionType


def _sub_ap(t, extra_offset, free_dims):
    """View of an SBUF/PSUM tile with custom (possibly overlapping) free dims."""
    base = t[:]
    return bass_rust.AP(
        base.tensor,
        base.offset + extra_offset,
        [list(base.ap[0])] + [list(d) for d in free_dims],
    )


def _ap(handle, offset, dims):
    """Arbitrary strided access pattern on a DRAM tensor handle."""
    return bass_rust.AP(handle, offset, [list(d) for d in dims])


def build_nc():
    nc = bacc.Bacc(num_devices=N_SECTIONS)

    xs8 = nc.dram_tensor("xs8", [LPRE + 1, FREQ], FP8, kind="ExternalInput")
    wdr = nc.dram_tensor("wdr", [128, 512], FP8, kind="ExternalInput")
    out = nc.dram_tensor("out", [1, 1], I32, kind="ExternalOutput")
    spk_dbg = nc.dram_tensor("spk_dbg", [N_CHANNELS, FP], F32, kind="ExternalOutput")
    cc_in = nc.dram_tensor("cc_in", [N_CHANNELS, FP], F32)
    cc_out = nc.dram_tensor(
        "cc_out", [N_SECTIONS, N_CHANNELS, FP], F32, addr_space="Shared"
    )

    with tile.TileContext(nc) as tc:
        with (
            tc.tile_pool(name="wp", bufs=1) as wp,
            tc.tile_pool(name="xp", bufs=6) as xp,
            tc.tile_pool(name="cp", bufs=4) as cp,
            tc.tile_pool(name="pp", bufs=6, space="PSUM") as pp,
            tc.tile_pool(name="pf", bufs=1, space="PSUM") as pf,
            tc.tile_pool(name="mp", bufs=1) as mpool,
        ):
            # ---- weights: SBUF [p=(dfc,dt)=128, (h, i, c)] fp8 ----
            wtile = wp.tile([128, 512], FP8)
            nc.sync.dma_start(out=wtile[:], in_=wdr[:])

            # ---- iota helpers (overlap with conv) ----
            # iomat[p, j] = p - j ; cidx[p] = p ; idn = (iomat == 0)
            iomat = mpool.tile([128, 128], F32)
            nc.gpsimd.iota(
                iomat[:], [[-1, 128]], base=0, channel_multiplier=1,
                allow_small_or_imprecise_dtypes=True,
            )
            idn = mpool.tile([128, 128], F32)
            nc.vector.tensor_single_scalar(idn[:], iomat[:], 0.0, OP.is_equal)

            # ---- running window maxes, q-major: macc[p, q*400 + t] ----
            macc = mpool.tile([128, FP * LPOST], BF16)

            xs8_h = xs8[:].tensor
            xpitch = None

            for b in range(N_BATCH):
                t0 = b * T_BATCH
                xr = xp.tile([128, XCOLS], FP8)
                # partition (dfc, dt) holds xs8[t0 + dt, dfc : dfc + XCOLS]
                src = _ap(
                    xs8_h,
                    t0 * FREQ,
                    [
                        (1, NDFC),        # dfc  (partition, outer)
                        (FREQ, KT),       # dt   (partition, inner)
                        (1, XCOLS),       # contiguous run (free)
                    ],
                )
                dma_eng = nc.sync if (b % 2 == 0) else nc.scalar
                dma_eng.dma_start(out=xr[:], in_=src)

                for pr in range(PAIRS_PER_BATCH):
                    pair = b * PAIRS_PER_BATCH + pr
                    ps = pp.tile([128, T_PAIR * FREQ], F32)
                    for h in range(2):
                        # rhs free dims: (i: shift 4, 2) x (cols: 512)
                        rhs = _sub_ap(
                            xr, pr * (T_PAIR * FREQ) + 8 * h,
                            [(4, 2), (1, T_PAIR * FREQ)],
                        )
                        lhsT = _sub_ap(wtile, 256 * h, [(128, 2), (1, 128)])
                        nc.tensor.matmul(
                            ps[:], lhsT, rhs,
                            start=(h == 0), stop=(h == 1), perf_mode=DR,
                        )
                    # windowed max [128, (q,tt,w)] -> macc[:, q*400 + pair*2 + tt]
                    dst = _sub_ap(
                        macc, pair * T_PAIR, [(LPOST, FP), (1, T_PAIR)]
                    )
                    if pr % 2 == 0:
                        # scalar copies PSUM -> SBUF bf16; vector reduces at 4x
                        sb = cp.tile([128, T_PAIR * FREQ], BF16)
                        nc.scalar.activation(sb[:], ps[:], AF.Copy)
                        src_r = _sub_ap(
                            sb, 0, [(KF, FP), (FREQ, T_PAIR), (1, KF)]
                        )
                    else:
                        # vector reduces straight from PSUM
                        src_r = _sub_ap(
                            ps, 0, [(KF, FP), (FREQ, T_PAIR), (1, KF)]
                        )
                    nc.vector.tensor_reduce(
                        dst, src_r, axis=mybir.AxisListType.X, op=OP.max
                    )

            # ---- final max over all 400 t per q, then threshold ----
            mpt = mpool.tile([128, FP], BF16)
            mview = _sub_ap(macc, 0, [(LPOST, FP), (1, LPOST)])
            nc.vector.tensor_reduce(
                mpt[:], mview, axis=mybir.AxisListType.X, op=OP.max
            )
            spk_loc = mpool.tile([128, FP], F32)
            nc.vector.tensor_single_scalar(spk_loc[:], mpt[:], THRESHOLD, OP.is_ge)
            nc.sync.dma_start(out=cc_in[:], in_=spk_loc[:])
            nc.sync.dma_start(out=spk_dbg[:], in_=spk_loc[:])

            # ---- all-gather binary spike maps across the 8 cores ----
            nc.gpsimd.collective_compute(
                "AllGather",
                OP.bypass,
                replica_groups=[list(range(N_SECTIONS))],
                ins=[cc_in[:]],
                outs=[cc_out[:]],
            )

            # ---- gather to SBUF: gt[c, (s, q)] (60B runs) ----
            gt = mpool.tile([128, N_SECTIONS * FP], F32)
            gsrc = _ap(
                cc_out[:].tensor,
                0,
                [
                    (FP, N_CHANNELS),             # c (partition)
                    (N_CHANNELS * FP, N_SECTIONS),  # s (free)
                    (1, FP),                      # q (free, contiguous)
                ],
            )
            nc.sync.dma_start(out=gt[:], in_=gsrc)

            # ---- per-(c,q) stats ----
            # n = sum_s spk ; e = min(8-n, 7) ; val = sum_s spk_s * (e == s)
            spk_qs = _sub_ap(gt, 0, [(1, FP), (FP, N_SECTIONS)])
            n_t = mpool.tile([128, FP], F32)
            nc.vector.tensor_reduce(
                n_t[:], spk_qs, axis=mybir.AxisListType.X, op=OP.add
            )
            e_t = mpool.tile([128, FP], F32)
            nc.vector.tensor_scalar(
                e_t[:], n_t[:], float(N_SECTIONS), -1.0, OP.subtract, OP.mult
            )
            nc.vector.tensor_scalar_min(e_t[:], e_t[:], float(N_SECTIONS - 1))

            val = mpool.tile([128, FP], F32)
            nc.vector.memset(val[:], 0.0)
            tmp = mpool.tile([128, FP], F32)
            for s in range(N_SECTIONS):
                nc.vector.scalar_tensor_tensor(
                    tmp[:], e_t[:], float(s), gt[:, s * FP : (s + 1) * FP],
                    OP.is_equal, OP.mult,
                )
                nc.vector.tensor_tensor(val[:], val[:], tmp[:], OP.add)

            # q_t = val * min(n, 1)  (for the global "any winner" test)
            nmin = mpool.tile([128, FP], F32)
            nc.vector.tensor_scalar_min(nmin[:], n_t[:], 1.0)
            q_t = mpool.tile([128, FP], F32)
            nc.vector.tensor_tensor(q_t[:], val[:], nmin[:], OP.mult)
            # tot = n * (val + 8)
            tot = mpool.tile([128, FP], F32)
            nc.vector.tensor_scalar_add(tot[:], val[:], float(N_SECTIONS))
            nc.vector.tensor_tensor(tot[:], tot[:], n_t[:], OP.mult)

            # per-channel maxima [128, 3]: (mq_col, gmax_col, packed_col)
            cols = mpool.tile([128, 3], F32)
            nc.vector.tensor_reduce(
                cols[:, 0:1], q_t[:], axis=mybir.AxisListType.X, op=OP.max
            )
            nc.vector.tensor_reduce(
                cols[:, 1:2], tot[:], axis=mybir.AxisListType.X, op=OP.max
            )
            # packed = 256 * rmax - c   (exact in f32; rmax integer <= 72)
            nc.vector.scalar_tensor_tensor(
                cols[:, 2:3], cols[:, 1:2], 256.0, iomat[:, 0:1],
                OP.mult, OP.subtract,
            )

            # transpose the 3 columns to rows via PE, then reduce across c
            scl = mpool.tile([1, 3], F32)
            for k in range(3):
                pst = pf.tile([1, 128], F32, tag="pt")
                nc.tensor.matmul(
                    pst[:], cols[:, k : k + 1], idn[:], start=True, stop=True
                )
                nc.vector.tensor_reduce(
                    scl[:, k : k + 1], pst[:], axis=mybir.AxisListType.X, op=OP.max
                )

            # feat = 256*gmax - pmax ; g = (mq > 0) ; ans = feat*g + g - 1
            feat = mpool.tile([1, 1], F32)
            nc.vector.scalar_tensor_tensor(
                feat[:], scl[:, 1:2], 256.0, scl[:, 2:3], OP.mult, OP.subtract
            )
            g_t = mpool.tile([1, 1], F32)
            nc.vector.tensor_single_scalar(g_t[:], scl[:, 0:1], 0.0, OP.is_gt)
            ansf = mpool.tile([1, 1], F32)
            nc.vector.tensor_tensor(ansf[:], feat[:], g_t[:], OP.mult)
            nc.vector.tensor_tensor(ansf[:], ansf[:], g_t[:], OP.add)
            nc.vector.tensor_scalar_sub(ansf[:], ansf[:], 1.0)
            ansi = mpool.tile([1, 1], I32)
            nc.vector.tensor_copy(ansi[:], ansf[:])
            nc.sync.dma_start(out=out[:], in_=ansi[:])

    nc.compile()
    return nc


def prep_inputs(X, W):
    """Host-side sharding + layout packing. Returns in_maps for 8 cores."""
    X = np.asarray(X, dtype=np.float32)
    W = np.asarray(W, dtype=np.float32)
    in_maps = []
    for s in range(N_SECTIONS):
        xs = np.zeros((LPRE + 1, FREQ), dtype=np.float32)
        xs[:LPRE] = X[s * SECTION_DISTANCE : s * SECTION_DISTANCE + LPRE]
        # wdr[dfc*32+dt, h*256 + i*128 + c] = W[s, c, 0, dt, 4*(2h+i)+dfc]
        w = W[s, :, 0]                      # [c, dt, df]
        w = w.transpose(2, 1, 0)            # [df, dt, c]
        w = w.reshape(2, 2, NDFC, KT, N_CHANNELS)   # [h, i, dfc, dt, c]
        w = w.transpose(2, 3, 0, 1, 4)      # [dfc, dt, h, i, c]
        wdr = np.ascontiguousarray(w).reshape(128, 512)
        in_maps.append(
            {
                "xs8": xs.astype(ml_dtypes.float8_e4m3),
                "wdr": wdr.astype(ml_dtypes.float8_e4m3),
            }
        )
    return in_maps


_NC_CACHE = {}


def run(X, W, trace=False, **kwargs):
    if "nc" not in _NC_CACHE:
        _NC_CACHE["nc"] = build_nc()
    nc = _NC_CACHE["nc"]
    in_maps = prep_inputs(X, W)
    res = run_bass_kernel_spmd(
        nc, in_maps, core_ids=list(range(N_SECTIONS)), trace=trace, **kwargs
    )
    return np.int32(res.results[0]["out"][0, 0]), res


def kernel(X, W):
    ans, _ = run(X, W)
    return ans


if __name__ == "__main__":
    X = np.random.rand(N_TIMESTEPS, FREQ).astype(np.float32) * 0.073
    W = (0.8 + 0.05 * np.random.randn(N_SECTIONS, N_CHANNELS, 1, KT, KF)).astype(
        np.float32
    )
    print(kernel(X, W))
